# revision 83
# baseline (speedup 1.0000x reference)
"""NerfExperts MoE kernel for Trainium2, expert-parallel over 8 NeuronCores.

Strategy: each of the 1024 points is routed to one of 100 experts
(~2.3MB of fp32 weights each -> memory bound).  Experts are sharded
across the 8 cores (13 slots per core), tokens dispatched on the host,
and each expert's weights stream from HBM exactly once.  Most weight
regions are fp8 e3m4 scaled by 64 (matmul runs fp8 lhsT x bf16 rhs; the
2^-6 descale is fused into the PSUM->SBUF bias-add on DVE); only the
precision-sensitive skip slab of w5 and the tiny heads stay bf16, which
lands the end-to-end error at ~1.5e-2 against the 2e-2 gate.  Weights
ship LAYER-MAJOR as one ~0.2-0.9MB DMA per region on the sync HWDGE
ring only (per-ring DMAs execute FIFO; descriptor generation is ~0.6us
per 128-row DMA; the scalar engine must stay free for ACT work), padded
to 128 partition rows (sub-128-row DMAs land on a single SDMA engine at
~27GB/s and stall the whole ring).  The stream saturates the ~358GB/s
per-core HBM limit with zero bubbles, and the stage pipeline trails the
arrival front.  Activations stay transposed ([feature, token]); experts
advance through the MLP in small lockstep waves (3 slots, per-wave
token capacity) sharing PSUM tiles; per-expert fp32 biases are applied
via stride-0 broadcast APs on DVE with relu fused as a second DVE op.
Harmonic-embedding sin AND cos phases come out of one matmul (ones-row
rhs + pi/2 lhsT row places cos at the 32-aligned row block), one
Cody-Waite range-reduction chain, and one Sin ACT; the Sigmoid ACT
table is preloaded right after (a lazy load is ~1.3us on the tail).

Embedding tile rows: points: sin 0:18, cos 32:50, xyz 50:53 (53 rows);
dirs: sin 0:12, cos 32:44, xyz 44:47 (47 rows).  Dead rows are zero in
the weight slabs, so garbage there is harmless.
"""

import numpy as np
import ml_dtypes

import concourse.bass as bass
import concourse.bacc as bacc
import concourse.mybir as mybir
import concourse.tile as tile
from concourse.bass_utils import run_bass_kernel_spmd

PI = float(np.pi)
N_CORES = 8
E = 100
NX, ND = 6, 4
CAP_MAX = 128  # max tokens per expert slot (keeps matmul N and PSUM in range)
NB = 21       # bias table columns per slot (9 stages x2, ba, bc0, bc1)

FP8_SCALE = 64.0
# regions stored as fp8 e3m4 (x64); the rest are bf16
FP8R = frozenset(["l0", "w1", "w2", "w3", "w4", "w5", "w6", "w7", "wi", "wc0",
                  "rays"])
# bf16 regions pre-scaled x64 because they share a PSUM accumulation with an
# fp8 region (stage 5: w5 main fp8 + skip bf16)
SCALED_BF16 = frozenset(["skip"]) if "w5" in FP8R else frozenset()

# region -> (rows, cols per slot); "hd" merges wa (cols 0:2) + wc1 (2:5).
# l0/skip/rays only use rows 0:53 / 0:47 but are padded to 128 rows: DMAs
# with <128 partition rows land on a single SDMA engine (~27GB/s) and
# serialize the whole FIFO ring behind them.
REGIONS = {
    "l0":   (128, 256),
    "w1":   (128, 512), "w2": (128, 512), "w3": (128, 512), "w4": (128, 512),
    "w5":   (128, 512), "skip": (128, 256),
    "w6":   (128, 512), "w7":   (128, 512),
    "wi":   (128, 512), "hd":   (128, 6),
    "wc0":  (128, 256), "rays": (128, 128),
}
# weight-DMA / consumption order (one DMA per region: descriptor generation
# on the sequencer costs ~0.6us per 128-row DMA, so fewer+larger wins)
REGION_ORDER = ["l0", "w1", "w2", "w3", "w4", "skip", "w5", "w6", "w7",
                "wi", "hd", "wc0", "rays"]


def _make_waves(nslot, C):
    gmax = max(1, min(512 // (2 * C), 3))
    nw = int(np.ceil(nslot / gmax))
    base = nslot // nw
    rem = nslot - base * nw
    sizes = [base + (1 if i < rem else 0) for i in range(nw)]
    waves, s0 = [], 0
    for g in sizes:
        waves.append((s0, s0 + g))
        s0 += g
    return waves


def _pack_expert(reg, bt, s, nslot, inputs, e, waves):
    """Fill slot s of the per-region fp32 arrays and bias table.

    Paired biases are stored j-major per wave: for stage lidx and wave
    (s0, s1), cols [lidx*2*nslot + 2*s0 : ... + 2*s1] hold
    [b_lo(s0..s1), b_hi(s0..s1)] so a wave's bias is one contiguous
    [128, 2g] block (3D-broadcastable along C).
    """
    n2 = 2 * nslot
    s0, s1 = next(w for w in waves if w[0] <= s < w[1])
    g = s1 - s0

    def set_b2(lidx, b):
        base = lidx * n2 + 2 * s0 + (s - s0)
        bt[:, base] = b[0:128]
        bt[:, base + g] = b[128:256]

    def emb_rows(dst, w, o, ncol, nsin):
        dst[0:nsin, o: o + ncol] = w[0:nsin]
        dst[32:32 + nsin, o: o + ncol] = w[nsin:2 * nsin]
        dst[32 + nsin: 32 + nsin + 3, o: o + ncol] = w[2 * nsin: 2 * nsin + 3]

    emb_rows(reg["l0"], inputs["w0"][e], s * 256, 256, 18)
    set_b2(0, inputs["b0"][e])
    w5 = inputs["w5"][e]
    emb_rows(reg["skip"], w5[256:295], s * 256, 256, 18)
    emb_rows(reg["rays"], inputs["wc0"][e][256:283], s * 128, 128, 12)
    for l in (1, 2, 3, 4, 6, 7):
        w = inputs[f"w{l}"][e]
        o = s * 512
        for k in (0, 1):
            reg[f"w{l}"][:, o + k * 256: o + (k + 1) * 256] = w[128 * k: 128 * (k + 1)]
        set_b2(l, inputs[f"b{l}"][e])
    o = s * 512
    for k in (0, 1):
        reg["w5"][:, o + k * 256: o + (k + 1) * 256] = w5[128 * k: 128 * (k + 1)]
    set_b2(5, inputs["b5"][e])
    wi = inputs["wi"][e]
    for k in (0, 1):
        reg["wi"][:, o + k * 256: o + (k + 1) * 256] = wi[128 * k: 128 * (k + 1)]
    set_b2(8, inputs["bi"][e])
    wa = inputs["wa"][e][:, 0]
    reg["hd"][:, s * 6] = wa[0:128]
    reg["hd"][:, s * 6 + 1] = wa[128:256]
    bt[0, 18 * nslot + s] = inputs["ba"][e][0]
    wc0 = inputs["wc0"][e]
    reg["wc0"][:, s * 256: s * 256 + 128] = wc0[0:128]
    reg["wc0"][:, s * 256 + 128: s * 256 + 256] = wc0[128:256]
    bt[:, 19 * nslot + s] = inputs["bc0"][e]
    reg["hd"][:, s * 6 + 3: s * 6 + 6] = inputs["wc1"][e]
    bt[0:3, 20 * nslot + s] = inputs["bc1"][e]


# ---------------------------------------------------------------------------
# Device program
# ---------------------------------------------------------------------------

def _build_program(Cs, nslot):
    waves = _make_waves(nslot, Cs[0])
    nw = len(waves)
    assert len(Cs) == nw
    wave_off = []
    o = 0
    for wv, (s0, s1) in enumerate(waves):
        wave_off.append(o)
        o += (s1 - s0) * Cs[wv]
    nall = o
    f32 = mybir.dt.float32
    bf16 = mybir.dt.bfloat16
    f8e3 = mybir.dt.float8e3
    Sin = mybir.ActivationFunctionType.Sin
    Sigmoid = mybir.ActivationFunctionType.Sigmoid
    Relu = mybir.ActivationFunctionType.Relu
    ADD = mybir.AluOpType.add
    SUB = mybir.AluOpType.subtract
    MUL = mybir.AluOpType.mult
    MAX = mybir.AluOpType.max
    MIN = mybir.AluOpType.min
    INV2PI = float(np.float32(1.0 / (2 * PI)))
    MAGIC = 12582912.0            # 1.5 * 2**23: forces round-to-int in fp32
    C1 = 6.28125                  # 2*pi high part, exact in fp32
    C2 = float(np.float32(2 * PI - 6.28125))
    CLAMP = 3.1415925             # just under pi (ACT Sin domain is [-pi, pi])
    HALF_PI = float(np.float32(PI / 2))
    DESC = float(1.0 / FP8_SCALE)

    def rdt(r):
        return f8e3 if r in FP8R else bf16

    nc = bacc.Bacc("TRN2", target_bir_lowering=False, debug=False)
    wt_d = {}
    for r in REGION_ORDER:
        rows, cols = REGIONS[r]
        wt_d[r] = nc.dram_tensor(f"wt_{r}", (rows, nslot * cols), rdt(r),
                                 kind="ExternalInput")
    bt_d = nc.dram_tensor("bt", (128, NB * nslot), f32, kind="ExternalInput")
    # sm3 rows 0:3 = xyz coords / freq matrix, row 3 = ones / pi*0.5 offsets:
    # cols [0:50 fx50 | 50:94 fd44 | 94:94+nall pts4 | ...+nall dirs4]
    sm3_d = nc.dram_tensor("sm3", (4, 94 + 2 * nall), f32, kind="ExternalInput")
    xyz_d = nc.dram_tensor("xyzb", (6, nall), bf16, kind="ExternalInput")
    al_d = nc.dram_tensor("alpha_out", (1, nall), f32, kind="ExternalOutput")
    co_d = nc.dram_tensor("color_out", (3, nall), f32, kind="ExternalOutput")

    with tile.TileContext(nc) as tc:
        with (
            tc.tile_pool(name="cp", bufs=1) as cp,
            tc.tile_pool(name="xp", bufs=2 * nw + 2) as xp,
            tc.tile_pool(name="psA", bufs=6, space=bass.MemorySpace.PSUM) as psA,
            tc.tile_pool(name="psB", bufs=2, space=bass.MemorySpace.PSUM) as psB,
        ):
            embP = cp.tile([53, nall], bf16)   # sin 0:18, cos 32:50, xyz 50:53
            embD = cp.tile([47, nall], bf16)   # sin 0:12, cos 32:44, xyz 44:47
            nc.vector.memset(embP[:], 0.0)
            nc.vector.memset(embD[:], 0.0)
            # small inputs: scalar ring first, then weight pieces on both rings
            sm3 = cp.tile([4, 94 + 2 * nall], f32)
            nc.scalar.dma_start(sm3[:], sm3_d.ap()[:])
            nc.scalar.dma_start(embP[50:53, :], xyz_d.ap()[0:3, :])
            nc.scalar.dma_start(embD[44:47, :], xyz_d.ap()[3:6, :])
            bt_sb = cp.tile([128, NB * nslot], f32)
            nc.scalar.dma_start(bt_sb[:], bt_d.ap()[:])
            fx_sb = sm3[:, 0:50]
            fd_sb = sm3[:, 50:94]
            pts_sb = sm3[:, 94:94 + nall]
            dir_sb = sm3[:, 94 + nall:94 + 2 * nall]
            warm = cp.tile([1, 8], f32)
            nc.vector.memset(warm[:], 0.0)

            # ---- weight pieces, consumption order, both rings balanced ----
            wts = {}
            for r in REGION_ORDER:
                rows, cols = REGIONS[r]
                wts[r] = cp.tile([rows, nslot * cols], rdt(r),
                                 name=f"wt_{r}", tag=f"wt_{r}")
            # all weight DMAs on the sync HWDGE ring, in consumption order:
            # the scalar engine must stay free for ACT work (Sin/Relu), and a
            # single InstDMACopy already spreads across all 16 SDMA engines.
            for r in REGION_ORDER:
                nc.sync.dma_start(wts[r][:], wt_d[r].ap()[:])

            def slab(r, s, lo, hi, wv, rows=128):
                cols = REGIONS[r][1]
                o = s * cols
                return wts[r][0:rows, o + lo: o + hi]

            alpha_sb = cp.tile([1, nall], f32)
            color_sb = cp.tile([3, nall], f32)

            # frequency expansion + range-reduced sin (single-step reduction:
            # max |round(x/2pi)| is ~16 here, so the fp32-2pi residual error
            # is ~3e-6 - well under bf16 noise)
            TWO_PI = float(np.float32(2 * PI))

            def reduce_sin(tsrc, rows, ncol):
                t1 = xp.tile([rows, ncol], f32, tag="vred")
                nc.vector.tensor_scalar(t1[:], tsrc, INV2PI, MAGIC, MUL, ADD)
                r = xp.tile([rows, ncol], f32, tag="vred")
                nc.vector.tensor_scalar(r[:], t1[:], MAGIC, None, SUB)
                rd = xp.tile([rows, ncol], f32, tag="vred")
                nc.vector.scalar_tensor_tensor(rd[:], r[:], -TWO_PI, tsrc, MUL, ADD)
                v = xp.tile([rows, ncol], f32, tag="vred")
                nc.vector.tensor_scalar(v[:], rd[:], CLAMP, -CLAMP, MIN, MAX)
                return v

            # one matmul per embedding emits BOTH phase blocks (rows 0:nsin
            # sin, rows 32:32+nsin cos as x+pi/2 via the ones-row of the rhs),
            # so a single reduce chain + single Sin ACT covers sin and cos.
            for lo in range(0, nall, 512):
                hi = min(nall, lo + 512)
                w_ = hi - lo
                for (rows, fmat, src, dst) in (
                    (50, fx_sb, pts_sb, embP),
                    (44, fd_sb, dir_sb, embD),
                ):
                    ep = psA.tile([rows, w_], f32, tag="mlp")
                    nc.tensor.matmul(ep[:], fmat[:, 0:rows], src[:, lo:hi],
                                     start=True, stop=True)
                    vs = reduce_sin(ep[:], rows, w_)
                    nc.scalar.activation(dst[0:rows, lo:hi], vs[:], Sin)
            # preload the Sigmoid ACT table AFTER the embedding Sins (its
            # ~1.3us table load must not delay them, but must run long before
            # the final sigmoid at the tail); reading embD forces the
            # scheduler to order it behind the Sins
            nc.scalar.activation(warm[:], embD[0:1, 0:8], Sigmoid)

            # ---- wave-lockstep MLP ----
            def bias2_bcast(lidx, s0, s1, C):
                g = s1 - s0
                ap = bt_sb[:, lidx * 2 * nslot + s0 * 2: lidx * 2 * nslot + s1 * 2]
                return ap.broadcast_to([128, 2 * g, C])

            def bias1_bcast(which, s0, s1, C, p=128, p0=0):
                g = s1 - s0
                ap = bt_sb[p0:p0 + p, which * nslot + s0: which * nslot + s1]
                return ap.broadcast_to([p, g, C])

            xs = [None] * nw
            its = [None] * nw
            cts = [None] * nw

            def mm_mid(r, ps, xin, s0, s1, wv, C):
                for i in range(s1 - s0):
                    s = s0 + i
                    for j in (0, 1):
                        pj = ps[:, j, i * C:(i + 1) * C]
                        nc.tensor.matmul(pj, slab(r, s, j * 128, j * 128 + 128, wv),
                                         xin[:, 0, i * C:(i + 1) * C],
                                         start=True, stop=False)
                        nc.tensor.matmul(pj, slab(r, s, 256 + j * 128, 256 + j * 128 + 128, wv),
                                         xin[:, 1, i * C:(i + 1) * C],
                                         start=False, stop=True)

            def move2(ps, lidx, s0, s1, wv, C, relu=True, fp8=False):
                g = s1 - s0
                xn = xp.tile([128, 2, g * C], bf16, tag="x")
                psv = ps[:].rearrange("p j (g c) -> p (j g) c", g=g)
                xnv = xn[:].rearrange("p j (g c) -> p (j g) c", g=g)
                if fp8:
                    nc.vector.scalar_tensor_tensor(
                        xnv, psv, DESC, bias2_bcast(lidx, s0, s1, C), MUL, ADD)
                else:
                    nc.vector.tensor_tensor(xnv, psv, bias2_bcast(lidx, s0, s1, C), ADD)
                if relu:
                    nc.vector.tensor_scalar(xn[:], xn[:], 0.0, None, MAX)
                return xn

            def emit_stage(wv, stage):
                s0, s1 = waves[wv]
                g = s1 - s0
                C = Cs[wv]
                wo = wave_off[wv]
                if stage == 0:  # L0
                    ps = psA.tile([128, 2, g * C], f32, tag="mlp")
                    for i in range(g):
                        s = s0 + i
                        sl = slice(wo + i * C, wo + (i + 1) * C)
                        for j in (0, 1):
                            nc.tensor.matmul(ps[:, j, i * C:(i + 1) * C],
                                             slab("l0", s, j * 128, j * 128 + 128, wv, rows=53),
                                             embP[0:53, sl],
                                             start=True, stop=True)
                    xs[wv] = move2(ps, 0, s0, s1, wv, C, fp8="l0" in FP8R)
                elif stage in (1, 2, 3, 4, 6, 7):
                    r = f"w{stage}"
                    ps = psA.tile([128, 2, g * C], f32, tag="mlp")
                    mm_mid(r, ps, xs[wv], s0, s1, wv, C)
                    xs[wv] = move2(ps, stage, s0, s1, wv, C, fp8=r in FP8R)
                elif stage == 5:
                    ps = psA.tile([128, 2, g * C], f32, tag="mlp")
                    xin = xs[wv]
                    for i in range(g):
                        s = s0 + i
                        sl = slice(wo + i * C, wo + (i + 1) * C)
                        for j in (0, 1):
                            pj = ps[:, j, i * C:(i + 1) * C]
                            nc.tensor.matmul(pj, slab("w5", s, j * 128, j * 128 + 128, wv),
                                             xin[:, 0, i * C:(i + 1) * C],
                                             start=True, stop=False)
                            nc.tensor.matmul(pj, slab("w5", s, 256 + j * 128, 256 + j * 128 + 128, wv),
                                             xin[:, 1, i * C:(i + 1) * C],
                                             start=False, stop=False)
                            nc.tensor.matmul(pj, slab("skip", s, j * 128, j * 128 + 128, wv, rows=53),
                                             embP[0:53, sl],
                                             start=False, stop=True)
                    xs[wv] = move2(ps, 5, s0, s1, wv, C, fp8="w5" in FP8R)
                elif stage == 8:  # wi -> inter (bias, no relu)
                    ps = psA.tile([128, 2, g * C], f32, tag="mlp")
                    mm_mid("wi", ps, xs[wv], s0, s1, wv, C)
                    its[wv] = move2(ps, 8, s0, s1, wv, C, relu=False, fp8="wi" in FP8R)
                elif stage == 9:  # wa -> alpha
                    pa = psB.tile([3, g * C], f32, tag="head")
                    xin = xs[wv]
                    for i in range(g):
                        s = s0 + i
                        nc.tensor.matmul(pa[0:1, i * C:(i + 1) * C],
                                         slab("hd", s, 0, 1, wv),
                                         xin[:, 0, i * C:(i + 1) * C],
                                         start=True, stop=False)
                        nc.tensor.matmul(pa[0:1, i * C:(i + 1) * C],
                                         slab("hd", s, 1, 2, wv),
                                         xin[:, 1, i * C:(i + 1) * C],
                                         start=False, stop=True)
                    av = alpha_sb[0:1, wo: wo + g * C].rearrange(
                        "p (g c) -> p g c", g=g)
                    pav = pa[0:1, :].rearrange("p (g c) -> p g c", g=g)
                    nc.vector.tensor_tensor(av, pav, bias1_bcast(18, s0, s1, C, p=1), ADD)
                elif stage == 10:  # wc0 + rays -> c (relu)
                    pc = psA.tile([128, g * C], f32, tag="mlp")
                    it = its[wv]
                    for i in range(g):
                        s = s0 + i
                        sl = slice(wo + i * C, wo + (i + 1) * C)
                        pj = pc[:, i * C:(i + 1) * C]
                        nc.tensor.matmul(pj, slab("wc0", s, 0, 128, wv),
                                         it[:, 0, i * C:(i + 1) * C],
                                         start=True, stop=False)
                        nc.tensor.matmul(pj, slab("wc0", s, 128, 256, wv),
                                         it[:, 1, i * C:(i + 1) * C],
                                         start=False, stop=False)
                        nc.tensor.matmul(pj, slab("rays", s, 0, 128, wv, rows=47),
                                         embD[0:47, sl],
                                         start=False, stop=True)
                    ct = xp.tile([128, g * C], bf16, tag="ct")
                    pcv = pc[:].rearrange("p (g c) -> p g c", g=g)
                    ctv = ct[:].rearrange("p (g c) -> p g c", g=g)
                    if "wc0" in FP8R:
                        nc.vector.scalar_tensor_tensor(
                            ctv, pcv, DESC, bias1_bcast(19, s0, s1, C), MUL, ADD)
                    else:
                        nc.vector.tensor_tensor(ctv, pcv, bias1_bcast(19, s0, s1, C), ADD)
                    nc.vector.tensor_scalar(ct[:], ct[:], 0.0, None, MAX)
                    cts[wv] = ct
                elif stage == 11:  # wc1 -> sigmoid color
                    pcol = psB.tile([3, g * C], f32, tag="head")
                    ct = cts[wv]
                    for i in range(g):
                        s = s0 + i
                        nc.tensor.matmul(pcol[:, i * C:(i + 1) * C],
                                         slab("hd", s, 3, 6, wv),
                                         ct[:, i * C:(i + 1) * C],
                                         start=True, stop=True)
                    ctmp = xp.tile([3, g * C], f32, tag="ctmp")
                    pv = pcol[:].rearrange("p (g c) -> p g c", g=g)
                    cv = ctmp[:].rearrange("p (g c) -> p g c", g=g)
                    nc.vector.tensor_tensor(
                        cv, pv, bias1_bcast(20, s0, s1, C, p=3), ADD)
                    nc.scalar.activation(color_sb[0:3, wo: wo + g * C],
                                         ctmp[:], Sigmoid)
                    # per-wave color DMA on the idle SYNC ring (not scalar:
                    # a dma_start there would sit in the ACT FIFO between
                    # sigmoids); earlier waves' receipts overlap later waves
                    nc.sync.dma_start(co_d.ap()[0:3, wo: wo + g * C],
                                      color_sb[0:3, wo: wo + g * C])

            # stage 9 (alpha head) before 8: it feeds the output directly and
            # only needs xs, so it must not gate the wi->wc0->wc1 chain
            for stage in (0, 1, 2, 3, 4, 5, 6, 7, 9, 8, 10, 11):
                for wv in range(nw):
                    emit_stage(wv, stage)

            nc.sync.dma_start(al_d.ap()[:], alpha_sb[:])


    nc.compile()
    return nc


_prog_cache = {}
_last_results = None


def _get_program(Cs, nslot):
    key = (tuple(Cs), nslot)
    if key not in _prog_cache:
        _prog_cache[key] = _build_program(tuple(Cs), nslot)
    return _prog_cache[key]


# ---------------------------------------------------------------------------
# Host wrapper
# ---------------------------------------------------------------------------

def kernel(**inputs):
    global _last_results
    inputs = {k: np.asarray(v) for k, v in inputs.items()}
    idx = inputs["index"].astype(np.int64)
    B = idx.shape[0]
    points = inputs["points"].astype(np.float32)
    dirs = inputs["directions"].astype(np.float32)

    # --- routing: split each expert's tokens into <=CAP_MAX chunks, round-
    # robin (sorted by size) over 8 cores ---
    tok = [np.nonzero(idx == e)[0] for e in range(E)]
    virt = []
    for e in range(E):
        t = tok[e]
        if len(t) == 0:
            continue
        for lo in range(0, len(t), CAP_MAX):
            virt.append((e, t[lo: lo + CAP_MAX]))
    if not virt:
        virt = [(0, np.zeros((0,), np.int64))]
    virt.sort(key=lambda v: -len(v[1]))
    nslot = max(1, int(np.ceil(len(virt) / N_CORES)))
    C0 = max(4, int(np.ceil(max(len(v[1]) for v in virt) / 4) * 4))

    core_slots = [[] for _ in range(N_CORES)]
    for i, v in enumerate(virt):
        core_slots[i % N_CORES].append(v)
    waves = _make_waves(nslot, C0)
    # per-wave capacity: slots are size-sorted, so wave w's max size is the
    # first slot of the wave on any core
    Cs = []
    for (s0, s1) in waves:
        mx = max((len(virt[i][1]) for i in range(s0 * N_CORES,
                 min(len(virt), s1 * N_CORES))), default=4)
        Cs.append(max(4, int(np.ceil(mx / 4) * 4)))
    slot_off = []
    o = 0
    for wv, (s0, s1) in enumerate(waves):
        for s in range(s0, s1):
            slot_off.append(o)
            o += Cs[wv]
    nall = o

    nc = _get_program(Cs, nslot)

    # fx50/fd44: rows 0:3 frequency matrix (sin block at cols 0:nsin, cos
    # block at cols 32:32+nsin), row 3 = pi/2 phase offset under the cos
    # block (paired with the ones-row of pts4/dirs4)
    fx = np.zeros((4, 50), np.float32)
    for c in range(3):
        for k in range(NX):
            fx[c, c * NX + k] = float(2 ** k)
            fx[c, 32 + c * NX + k] = float(2 ** k)
    fx[3, 32:50] = float(np.float32(np.pi / 2))
    fd = np.zeros((4, 44), np.float32)
    for c in range(3):
        for k in range(ND):
            fd[c, c * ND + k] = float(2 ** k)
            fd[c, 32 + c * ND + k] = float(2 ** k)
    fd[3, 32:44] = float(np.float32(np.pi / 2))

    in_maps = []
    for c in range(N_CORES):
        reg = {r: np.zeros((REGIONS[r][0], nslot * REGIONS[r][1]), np.float32)
               for r in REGION_ORDER}
        bt = np.zeros((128, NB * nslot), np.float32)
        ptsT = np.zeros((3, nall), np.float32)
        dirT = np.zeros((3, nall), np.float32)
        for s, (e, t) in enumerate(core_slots[c]):
            _pack_expert(reg, bt, s, nslot, inputs, e, waves)
            n = len(t)
            if n:
                ptsT[:, slot_off[s]: slot_off[s] + n] = points[t].T
                dirT[:, slot_off[s]: slot_off[s] + n] = dirs[t].T
        ones = np.ones((1, nall), np.float32)
        pts4 = np.concatenate([ptsT, ones], axis=0)
        dirs4 = np.concatenate([dirT, ones], axis=0)
        sm3 = np.concatenate([fx, fd, pts4, dirs4], axis=1)
        xyzb = np.concatenate([ptsT, dirT], axis=0).astype(ml_dtypes.bfloat16)
        im = {"bt": bt, "sm3": sm3, "xyzb": xyzb}
        for r in REGION_ORDER:
            if r in FP8R:
                im[f"wt_{r}"] = (reg[r] * FP8_SCALE).astype(ml_dtypes.float8_e3m4)
            elif r in SCALED_BF16:
                im[f"wt_{r}"] = (reg[r] * FP8_SCALE).astype(ml_dtypes.bfloat16)
            else:
                im[f"wt_{r}"] = reg[r].astype(ml_dtypes.bfloat16)
        in_maps.append(im)

    res = run_bass_kernel_spmd(nc, in_maps, core_ids=list(range(N_CORES)))
    _last_results = res

    out = np.zeros((B, 4), np.float32)
    for c in range(N_CORES):
        al = res.results[c]["alpha_out"]
        co = res.results[c]["color_out"]
        for s, (e, t) in enumerate(core_slots[c]):
            n = len(t)
            if n:
                out[t, 0] = al[0, slot_off[s]: slot_off[s] + n]
                out[t, 1:4] = co[:, slot_off[s]: slot_off[s] + n].T
    return out


# revision 85
# speedup vs baseline: 1.0308x; 1.0308x over previous
"""NerfExperts MoE kernel for Trainium2, expert-parallel over 8 NeuronCores.

Strategy: each of the 1024 points is routed to one of 100 experts
(~2.3MB of fp32 weights each -> memory bound).  Experts are sharded
across the 8 cores (13 slots per core), tokens dispatched on the host,
and each expert's weights stream from HBM exactly once.  Most weight
regions are fp8 e3m4 scaled by 64 (matmul runs fp8 lhsT x bf16 rhs; the
2^-6 descale is fused into the PSUM->SBUF bias-add on DVE); only the
precision-sensitive skip slab of w5 and the tiny heads stay bf16, which
lands the end-to-end error at ~1.5e-2 against the 2e-2 gate.  Weights
ship LAYER-MAJOR as one ~0.2-0.9MB DMA per region on the sync HWDGE
ring only (per-ring DMAs execute FIFO; descriptor generation is ~0.6us
per 128-row DMA; the scalar engine must stay free for ACT work), padded
to 128 partition rows (sub-128-row DMAs land on a single SDMA engine at
~27GB/s and stall the whole ring).  The stream saturates the ~358GB/s
per-core HBM limit with zero bubbles, and the stage pipeline trails the
arrival front.  Activations stay transposed ([feature, token]); experts
advance through the MLP in small lockstep waves (3 slots, per-wave
token capacity) sharing PSUM tiles; per-expert fp32 biases are applied
via stride-0 broadcast APs on DVE with relu fused as a second DVE op.
Harmonic-embedding sin AND cos phases come out of one matmul (ones-row
rhs + pi/2 lhsT row places cos at the 32-aligned row block), one
Cody-Waite range-reduction chain, and one Sin ACT; the Sigmoid ACT
table is preloaded right after (a lazy load is ~1.3us on the tail).

Embedding tile rows: points: sin 0:18, cos 32:50, xyz 50:53 (53 rows);
dirs: sin 0:12, cos 32:44, xyz 44:47 (47 rows).  Dead rows are zero in
the weight slabs, so garbage there is harmless.
"""

import numpy as np
import ml_dtypes

import concourse.bass as bass
import concourse.bacc as bacc
import concourse.mybir as mybir
import concourse.tile as tile
from concourse.bass_utils import run_bass_kernel_spmd

PI = float(np.pi)
N_CORES = 8
E = 100
NX, ND = 6, 4
CAP_MAX = 128  # max tokens per expert slot (keeps matmul N and PSUM in range)
NB = 21       # bias table columns per slot (9 stages x2, ba, bc0, bc1)

FP8_SCALE = 64.0
# regions stored as fp8 e3m4 (x64); the rest are bf16
FP8R = frozenset(["l0", "w1", "w2", "w3", "w4", "w5", "w6", "w7", "wi", "wc0",
                  "rays"])
# bf16 regions pre-scaled x64 because they share a PSUM accumulation with an
# fp8 region (stage 5: w5 main fp8 + skip bf16)
SCALED_BF16 = frozenset(["skip"]) if "w5" in FP8R else frozenset()

# region -> (rows, cols per slot); "hd" merges wa (cols 0:2) + wc1 (2:5).
# l0/skip/rays only use rows 0:53 / 0:47 but are padded to 128 rows: DMAs
# with <128 partition rows land on a single SDMA engine (~27GB/s) and
# serialize the whole FIFO ring behind them.
REGIONS = {
    "l0":   (128, 256),
    "w1":   (128, 512), "w2": (128, 512), "w3": (128, 512), "w4": (128, 512),
    "w5":   (128, 512), "skip": (128, 256),
    "w6":   (128, 512), "w7":   (128, 512),
    "wi":   (128, 512), "hd":   (128, 6),
    "wc0":  (128, 256), "rays": (128, 128),
}
# weight-DMA / consumption order (one DMA per region: descriptor generation
# on the sequencer costs ~0.6us per 128-row DMA, so fewer+larger wins)
REGION_ORDER = ["l0", "w1", "w2", "w3", "w4", "skip", "w5", "w6", "w7",
                "wi", "hd", "wc0", "rays"]


def _make_waves(nslot, C):
    gmax = max(1, min(512 // (2 * C), 3))
    nw = int(np.ceil(nslot / gmax))
    base = nslot // nw
    rem = nslot - base * nw
    sizes = [base + (1 if i < rem else 0) for i in range(nw)]
    waves, s0 = [], 0
    for g in sizes:
        waves.append((s0, s0 + g))
        s0 += g
    return waves


def _pack_expert(reg, bt, s, nslot, inputs, e, waves):
    """Fill slot s of the per-region fp32 arrays and bias table.

    Paired biases are stored j-major per wave: for stage lidx and wave
    (s0, s1), cols [lidx*2*nslot + 2*s0 : ... + 2*s1] hold
    [b_lo(s0..s1), b_hi(s0..s1)] so a wave's bias is one contiguous
    [128, 2g] block (3D-broadcastable along C).
    """
    n2 = 2 * nslot
    s0, s1 = next(w for w in waves if w[0] <= s < w[1])
    g = s1 - s0

    def set_b2(lidx, b):
        base = lidx * n2 + 2 * s0 + (s - s0)
        bt[:, base] = b[0:128]
        bt[:, base + g] = b[128:256]

    def emb_rows(dst, w, o, ncol, nsin):
        dst[0:nsin, o: o + ncol] = w[0:nsin]
        dst[32:32 + nsin, o: o + ncol] = w[nsin:2 * nsin]
        dst[32 + nsin: 32 + nsin + 3, o: o + ncol] = w[2 * nsin: 2 * nsin + 3]

    emb_rows(reg["l0"], inputs["w0"][e], s * 256, 256, 18)
    set_b2(0, inputs["b0"][e])
    w5 = inputs["w5"][e]
    emb_rows(reg["skip"], w5[256:295], s * 256, 256, 18)
    emb_rows(reg["rays"], inputs["wc0"][e][256:283], s * 128, 128, 12)
    for l in (1, 2, 3, 4, 6, 7):
        w = inputs[f"w{l}"][e]
        o = s * 512
        for k in (0, 1):
            reg[f"w{l}"][:, o + k * 256: o + (k + 1) * 256] = w[128 * k: 128 * (k + 1)]
        set_b2(l, inputs[f"b{l}"][e])
    o = s * 512
    for k in (0, 1):
        reg["w5"][:, o + k * 256: o + (k + 1) * 256] = w5[128 * k: 128 * (k + 1)]
    set_b2(5, inputs["b5"][e])
    wi = inputs["wi"][e]
    for k in (0, 1):
        reg["wi"][:, o + k * 256: o + (k + 1) * 256] = wi[128 * k: 128 * (k + 1)]
    set_b2(8, inputs["bi"][e])
    wa = inputs["wa"][e][:, 0]
    reg["hd"][:, s * 6] = wa[0:128]
    reg["hd"][:, s * 6 + 1] = wa[128:256]
    bt[0, 18 * nslot + s] = inputs["ba"][e][0]
    wc0 = inputs["wc0"][e]
    reg["wc0"][:, s * 256: s * 256 + 128] = wc0[0:128]
    reg["wc0"][:, s * 256 + 128: s * 256 + 256] = wc0[128:256]
    bt[:, 19 * nslot + s] = inputs["bc0"][e]
    reg["hd"][:, s * 6 + 3: s * 6 + 6] = inputs["wc1"][e]
    bt[0:3, 20 * nslot + s] = inputs["bc1"][e]


# ---------------------------------------------------------------------------
# Device program
# ---------------------------------------------------------------------------

def _build_program(Cs, nslot):
    waves = _make_waves(nslot, Cs[0])
    nw = len(waves)
    assert len(Cs) == nw
    wave_off = []
    o = 0
    for wv, (s0, s1) in enumerate(waves):
        wave_off.append(o)
        o += (s1 - s0) * Cs[wv]
    nall = o
    f32 = mybir.dt.float32
    bf16 = mybir.dt.bfloat16
    f8e3 = mybir.dt.float8e3
    Sin = mybir.ActivationFunctionType.Sin
    Sigmoid = mybir.ActivationFunctionType.Sigmoid
    Relu = mybir.ActivationFunctionType.Relu
    ADD = mybir.AluOpType.add
    SUB = mybir.AluOpType.subtract
    MUL = mybir.AluOpType.mult
    MAX = mybir.AluOpType.max
    MIN = mybir.AluOpType.min
    INV2PI = float(np.float32(1.0 / (2 * PI)))
    MAGIC = 12582912.0            # 1.5 * 2**23: forces round-to-int in fp32
    C1 = 6.28125                  # 2*pi high part, exact in fp32
    C2 = float(np.float32(2 * PI - 6.28125))
    CLAMP = 3.1415925             # just under pi (ACT Sin domain is [-pi, pi])
    HALF_PI = float(np.float32(PI / 2))
    DESC = float(1.0 / FP8_SCALE)

    def rdt(r):
        return f8e3 if r in FP8R else bf16

    nc = bacc.Bacc("TRN2", target_bir_lowering=False, debug=False)
    wt_d = {}
    for r in REGION_ORDER:
        rows, cols = REGIONS[r]
        wt_d[r] = nc.dram_tensor(f"wt_{r}", (rows, nslot * cols), rdt(r),
                                 kind="ExternalInput")
    bt_d = nc.dram_tensor("bt", (128, NB * nslot), f32, kind="ExternalInput")
    # sm3 rows 0:3 = xyz coords / freq matrix, row 3 = ones / pi*0.5 offsets:
    # cols [0:50 fx50 | 50:94 fd44 | 94:94+nall pts4 | ...+nall dirs4]
    sm3_d = nc.dram_tensor("sm3", (4, 94 + 2 * nall), f32, kind="ExternalInput")
    xyz_d = nc.dram_tensor("xyzb", (6, nall), bf16, kind="ExternalInput")
    al_d = nc.dram_tensor("alpha_out", (1, nall), f32, kind="ExternalOutput")
    co_d = nc.dram_tensor("color_out", (3, nall), f32, kind="ExternalOutput")

    with tile.TileContext(nc) as tc:
        with (
            tc.tile_pool(name="cp", bufs=1) as cp,
            tc.tile_pool(name="xp", bufs=2 * nw + 2) as xp,
            tc.tile_pool(name="psA", bufs=6, space=bass.MemorySpace.PSUM) as psA,
            tc.tile_pool(name="psB", bufs=2, space=bass.MemorySpace.PSUM) as psB,
        ):
            embP = cp.tile([53, nall], bf16)   # sin 0:18, cos 32:50, xyz 50:53
            embD = cp.tile([47, nall], bf16)   # sin 0:12, cos 32:44, xyz 44:47
            nc.vector.memset(embP[:], 0.0)
            nc.vector.memset(embD[:], 0.0)
            # small inputs: scalar ring first, then weight pieces on both rings
            sm3 = cp.tile([4, 94 + 2 * nall], f32)
            nc.scalar.dma_start(sm3[:], sm3_d.ap()[:])
            nc.scalar.dma_start(embP[50:53, :], xyz_d.ap()[0:3, :])
            nc.scalar.dma_start(embD[44:47, :], xyz_d.ap()[3:6, :])
            bt_sb = cp.tile([128, NB * nslot], f32)
            nc.scalar.dma_start(bt_sb[:], bt_d.ap()[:])
            fx_sb = sm3[:, 0:50]
            fd_sb = sm3[:, 50:94]
            pts_sb = sm3[:, 94:94 + nall]
            dir_sb = sm3[:, 94 + nall:94 + 2 * nall]
            warm = cp.tile([1, 8], f32)
            nc.vector.memset(warm[:], 0.0)

            # ---- weight pieces, consumption order, both rings balanced ----
            wts = {}
            for r in REGION_ORDER:
                rows, cols = REGIONS[r]
                wts[r] = cp.tile([rows, nslot * cols], rdt(r),
                                 name=f"wt_{r}", tag=f"wt_{r}")
            # all weight DMAs on the sync HWDGE ring, in consumption order:
            # the scalar engine must stay free for ACT work (Sin/Relu), and a
            # single InstDMACopy already spreads across all 16 SDMA engines.
            for r in REGION_ORDER:
                nc.sync.dma_start(wts[r][:], wt_d[r].ap()[:])

            def slab(r, s, lo, hi, wv, rows=128):
                cols = REGIONS[r][1]
                o = s * cols
                return wts[r][0:rows, o + lo: o + hi]

            alpha_sb = cp.tile([1, nall], f32)
            color_sb = cp.tile([3, nall], f32)

            # frequency expansion + range-reduced sin (single-step reduction:
            # max |round(x/2pi)| is ~16 here, so the fp32-2pi residual error
            # is ~3e-6 - well under bf16 noise)
            TWO_PI = float(np.float32(2 * PI))

            def reduce_sin(tsrc, rows, ncol):
                t1 = xp.tile([rows, ncol], f32, tag="vred")
                nc.vector.tensor_scalar(t1[:], tsrc, INV2PI, MAGIC, MUL, ADD)
                r = xp.tile([rows, ncol], f32, tag="vred")
                nc.vector.tensor_scalar(r[:], t1[:], MAGIC, None, SUB)
                rd = xp.tile([rows, ncol], f32, tag="vred")
                nc.vector.scalar_tensor_tensor(rd[:], r[:], -TWO_PI, tsrc, MUL, ADD)
                v = xp.tile([rows, ncol], f32, tag="vred")
                nc.vector.tensor_scalar(v[:], rd[:], CLAMP, -CLAMP, MIN, MAX)
                return v

            # one matmul per embedding emits BOTH phase blocks (rows 0:nsin
            # sin, rows 32:32+nsin cos as x+pi/2 via the ones-row of the rhs),
            # so a single reduce chain + single Sin ACT covers sin and cos.
            for lo in range(0, nall, 512):
                hi = min(nall, lo + 512)
                w_ = hi - lo
                for (rows, fmat, src, dst) in (
                    (50, fx_sb, pts_sb, embP),
                    (44, fd_sb, dir_sb, embD),
                ):
                    ep = psA.tile([rows, w_], f32, tag="mlp")
                    nc.tensor.matmul(ep[:], fmat[:, 0:rows], src[:, lo:hi],
                                     start=True, stop=True)
                    vs = reduce_sin(ep[:], rows, w_)
                    nc.scalar.activation(dst[0:rows, lo:hi], vs[:], Sin)
            # preload the Sigmoid ACT table AFTER the embedding Sins (its
            # ~1.3us table load must not delay them, but must run long before
            # the final sigmoid at the tail); reading embD forces the
            # scheduler to order it behind the Sins
            nc.scalar.activation(warm[:], embD[0:1, 0:8], Sigmoid)

            # ---- wave-lockstep MLP ----
            def bias2_bcast(lidx, s0, s1, C):
                g = s1 - s0
                ap = bt_sb[:, lidx * 2 * nslot + s0 * 2: lidx * 2 * nslot + s1 * 2]
                return ap.broadcast_to([128, 2 * g, C])

            def bias1_bcast(which, s0, s1, C, p=128, p0=0):
                g = s1 - s0
                ap = bt_sb[p0:p0 + p, which * nslot + s0: which * nslot + s1]
                return ap.broadcast_to([p, g, C])

            xs = [None] * nw
            its = [None] * nw
            cts = [None] * nw

            def mm_mid(r, ps, xin, s0, s1, wv, C):
                for i in range(s1 - s0):
                    s = s0 + i
                    for j in (0, 1):
                        pj = ps[:, j, i * C:(i + 1) * C]
                        nc.tensor.matmul(pj, slab(r, s, j * 128, j * 128 + 128, wv),
                                         xin[:, 0, i * C:(i + 1) * C],
                                         start=True, stop=False)
                        nc.tensor.matmul(pj, slab(r, s, 256 + j * 128, 256 + j * 128 + 128, wv),
                                         xin[:, 1, i * C:(i + 1) * C],
                                         start=False, stop=True)

            def move2(ps, lidx, s0, s1, wv, C, relu=True, fp8=False):
                g = s1 - s0
                xn = xp.tile([128, 2, g * C], bf16, tag="x")
                psv = ps[:].rearrange("p j (g c) -> p (j g) c", g=g)
                xnv = xn[:].rearrange("p j (g c) -> p (j g) c", g=g)
                if fp8:
                    nc.vector.scalar_tensor_tensor(
                        xnv, psv, DESC, bias2_bcast(lidx, s0, s1, C), MUL, ADD)
                else:
                    nc.vector.tensor_tensor(xnv, psv, bias2_bcast(lidx, s0, s1, C), ADD)
                if relu:
                    nc.vector.tensor_scalar(xn[:], xn[:], 0.0, None, MAX)
                return xn

            def emit_stage(wv, stage):
                s0, s1 = waves[wv]
                g = s1 - s0
                C = Cs[wv]
                wo = wave_off[wv]
                if stage == 0:  # L0
                    ps = psA.tile([128, 2, g * C], f32, tag="mlp")
                    for i in range(g):
                        s = s0 + i
                        sl = slice(wo + i * C, wo + (i + 1) * C)
                        for j in (0, 1):
                            nc.tensor.matmul(ps[:, j, i * C:(i + 1) * C],
                                             slab("l0", s, j * 128, j * 128 + 128, wv, rows=53),
                                             embP[0:53, sl],
                                             start=True, stop=True)
                    xs[wv] = move2(ps, 0, s0, s1, wv, C, fp8="l0" in FP8R)
                elif stage in (1, 2, 3, 4, 6, 7):
                    r = f"w{stage}"
                    ps = psA.tile([128, 2, g * C], f32, tag="mlp")
                    mm_mid(r, ps, xs[wv], s0, s1, wv, C)
                    xs[wv] = move2(ps, stage, s0, s1, wv, C, fp8=r in FP8R)
                elif stage == 5:
                    ps = psA.tile([128, 2, g * C], f32, tag="mlp")
                    xin = xs[wv]
                    for i in range(g):
                        s = s0 + i
                        sl = slice(wo + i * C, wo + (i + 1) * C)
                        for j in (0, 1):
                            pj = ps[:, j, i * C:(i + 1) * C]
                            nc.tensor.matmul(pj, slab("w5", s, j * 128, j * 128 + 128, wv),
                                             xin[:, 0, i * C:(i + 1) * C],
                                             start=True, stop=False)
                            nc.tensor.matmul(pj, slab("w5", s, 256 + j * 128, 256 + j * 128 + 128, wv),
                                             xin[:, 1, i * C:(i + 1) * C],
                                             start=False, stop=False)
                            nc.tensor.matmul(pj, slab("skip", s, j * 128, j * 128 + 128, wv, rows=53),
                                             embP[0:53, sl],
                                             start=False, stop=True)
                    xs[wv] = move2(ps, 5, s0, s1, wv, C, fp8="w5" in FP8R)
                elif stage == 8:  # wi -> inter (bias, no relu)
                    ps = psA.tile([128, 2, g * C], f32, tag="mlp")
                    mm_mid("wi", ps, xs[wv], s0, s1, wv, C)
                    its[wv] = move2(ps, 8, s0, s1, wv, C, relu=False, fp8="wi" in FP8R)
                elif stage == 9:  # wa -> alpha
                    pa = psB.tile([3, g * C], f32, tag="head")
                    xin = xs[wv]
                    for i in range(g):
                        s = s0 + i
                        nc.tensor.matmul(pa[0:1, i * C:(i + 1) * C],
                                         slab("hd", s, 0, 1, wv),
                                         xin[:, 0, i * C:(i + 1) * C],
                                         start=True, stop=False)
                        nc.tensor.matmul(pa[0:1, i * C:(i + 1) * C],
                                         slab("hd", s, 1, 2, wv),
                                         xin[:, 1, i * C:(i + 1) * C],
                                         start=False, stop=True)
                    av = alpha_sb[0:1, wo: wo + g * C].rearrange(
                        "p (g c) -> p g c", g=g)
                    pav = pa[0:1, :].rearrange("p (g c) -> p g c", g=g)
                    nc.vector.tensor_tensor(av, pav, bias1_bcast(18, s0, s1, C, p=1), ADD)
                elif stage == 10:  # wc0 + rays -> c (relu)
                    pc = psA.tile([128, g * C], f32, tag="mlp")
                    it = its[wv]
                    for i in range(g):
                        s = s0 + i
                        sl = slice(wo + i * C, wo + (i + 1) * C)
                        pj = pc[:, i * C:(i + 1) * C]
                        nc.tensor.matmul(pj, slab("wc0", s, 0, 128, wv),
                                         it[:, 0, i * C:(i + 1) * C],
                                         start=True, stop=False)
                        nc.tensor.matmul(pj, slab("wc0", s, 128, 256, wv),
                                         it[:, 1, i * C:(i + 1) * C],
                                         start=False, stop=False)
                        nc.tensor.matmul(pj, slab("rays", s, 0, 128, wv, rows=47),
                                         embD[0:47, sl],
                                         start=False, stop=True)
                    ct = xp.tile([128, g * C], bf16, tag="ct")
                    pcv = pc[:].rearrange("p (g c) -> p g c", g=g)
                    ctv = ct[:].rearrange("p (g c) -> p g c", g=g)
                    if "wc0" in FP8R:
                        nc.vector.scalar_tensor_tensor(
                            ctv, pcv, DESC, bias1_bcast(19, s0, s1, C), MUL, ADD)
                    else:
                        nc.vector.tensor_tensor(ctv, pcv, bias1_bcast(19, s0, s1, C), ADD)
                    nc.vector.tensor_scalar(ct[:], ct[:], 0.0, None, MAX)
                    cts[wv] = ct
                elif stage == 11:  # wc1 -> sigmoid color
                    pcol = psB.tile([3, g * C], f32, tag="head")
                    ct = cts[wv]
                    for i in range(g):
                        s = s0 + i
                        nc.tensor.matmul(pcol[:, i * C:(i + 1) * C],
                                         slab("hd", s, 3, 6, wv),
                                         ct[:, i * C:(i + 1) * C],
                                         start=True, stop=True)
                    ctmp = xp.tile([3, g * C], f32, tag="ctmp")
                    pv = pcol[:].rearrange("p (g c) -> p g c", g=g)
                    cv = ctmp[:].rearrange("p (g c) -> p g c", g=g)
                    nc.vector.tensor_tensor(
                        cv, pv, bias1_bcast(20, s0, s1, C, p=3), ADD)
                    nc.scalar.activation(color_sb[0:3, wo: wo + g * C],
                                         ctmp[:], Sigmoid)

            # stage 9 (alpha head) before 8: it feeds the output directly and
            # only needs xs, so it must not gate the wi->wc0->wc1 chain
            for stage in (0, 1, 2, 3, 4, 5, 6, 7, 9, 8, 10, 11):
                for wv in range(nw):
                    emit_stage(wv, stage)

            nc.sync.dma_start(al_d.ap()[:], alpha_sb[:])
            nc.scalar.dma_start(co_d.ap()[:], color_sb[:])


    nc.compile()
    return nc


_prog_cache = {}
_last_results = None


def _get_program(Cs, nslot):
    key = (tuple(Cs), nslot)
    if key not in _prog_cache:
        _prog_cache[key] = _build_program(tuple(Cs), nslot)
    return _prog_cache[key]


# ---------------------------------------------------------------------------
# Host wrapper
# ---------------------------------------------------------------------------

def kernel(**inputs):
    global _last_results
    inputs = {k: np.asarray(v) for k, v in inputs.items()}
    idx = inputs["index"].astype(np.int64)
    B = idx.shape[0]
    points = inputs["points"].astype(np.float32)
    dirs = inputs["directions"].astype(np.float32)

    # --- routing: split each expert's tokens into <=CAP_MAX chunks, round-
    # robin (sorted by size) over 8 cores ---
    tok = [np.nonzero(idx == e)[0] for e in range(E)]
    virt = []
    for e in range(E):
        t = tok[e]
        if len(t) == 0:
            continue
        for lo in range(0, len(t), CAP_MAX):
            virt.append((e, t[lo: lo + CAP_MAX]))
    if not virt:
        virt = [(0, np.zeros((0,), np.int64))]
    virt.sort(key=lambda v: -len(v[1]))
    nslot = max(1, int(np.ceil(len(virt) / N_CORES)))
    C0 = max(4, int(np.ceil(max(len(v[1]) for v in virt) / 4) * 4))

    core_slots = [[] for _ in range(N_CORES)]
    for i, v in enumerate(virt):
        core_slots[i % N_CORES].append(v)
    waves = _make_waves(nslot, C0)
    # per-wave capacity: slots are size-sorted, so wave w's max size is the
    # first slot of the wave on any core
    Cs = []
    for (s0, s1) in waves:
        mx = max((len(virt[i][1]) for i in range(s0 * N_CORES,
                 min(len(virt), s1 * N_CORES))), default=4)
        Cs.append(max(4, int(np.ceil(mx / 4) * 4)))
    slot_off = []
    o = 0
    for wv, (s0, s1) in enumerate(waves):
        for s in range(s0, s1):
            slot_off.append(o)
            o += Cs[wv]
    nall = o

    nc = _get_program(Cs, nslot)

    # fx50/fd44: rows 0:3 frequency matrix (sin block at cols 0:nsin, cos
    # block at cols 32:32+nsin), row 3 = pi/2 phase offset under the cos
    # block (paired with the ones-row of pts4/dirs4)
    fx = np.zeros((4, 50), np.float32)
    for c in range(3):
        for k in range(NX):
            fx[c, c * NX + k] = float(2 ** k)
            fx[c, 32 + c * NX + k] = float(2 ** k)
    fx[3, 32:50] = float(np.float32(np.pi / 2))
    fd = np.zeros((4, 44), np.float32)
    for c in range(3):
        for k in range(ND):
            fd[c, c * ND + k] = float(2 ** k)
            fd[c, 32 + c * ND + k] = float(2 ** k)
    fd[3, 32:44] = float(np.float32(np.pi / 2))

    in_maps = []
    for c in range(N_CORES):
        reg = {r: np.zeros((REGIONS[r][0], nslot * REGIONS[r][1]), np.float32)
               for r in REGION_ORDER}
        bt = np.zeros((128, NB * nslot), np.float32)
        ptsT = np.zeros((3, nall), np.float32)
        dirT = np.zeros((3, nall), np.float32)
        for s, (e, t) in enumerate(core_slots[c]):
            _pack_expert(reg, bt, s, nslot, inputs, e, waves)
            n = len(t)
            if n:
                ptsT[:, slot_off[s]: slot_off[s] + n] = points[t].T
                dirT[:, slot_off[s]: slot_off[s] + n] = dirs[t].T
        ones = np.ones((1, nall), np.float32)
        pts4 = np.concatenate([ptsT, ones], axis=0)
        dirs4 = np.concatenate([dirT, ones], axis=0)
        sm3 = np.concatenate([fx, fd, pts4, dirs4], axis=1)
        xyzb = np.concatenate([ptsT, dirT], axis=0).astype(ml_dtypes.bfloat16)
        im = {"bt": bt, "sm3": sm3, "xyzb": xyzb}
        for r in REGION_ORDER:
            if r in FP8R:
                im[f"wt_{r}"] = (reg[r] * FP8_SCALE).astype(ml_dtypes.float8_e3m4)
            elif r in SCALED_BF16:
                im[f"wt_{r}"] = (reg[r] * FP8_SCALE).astype(ml_dtypes.bfloat16)
            else:
                im[f"wt_{r}"] = reg[r].astype(ml_dtypes.bfloat16)
        in_maps.append(im)

    res = run_bass_kernel_spmd(nc, in_maps, core_ids=list(range(N_CORES)))
    _last_results = res

    out = np.zeros((B, 4), np.float32)
    for c in range(N_CORES):
        al = res.results[c]["alpha_out"]
        co = res.results[c]["color_out"]
        for s, (e, t) in enumerate(core_slots[c]):
            n = len(t)
            if n:
                out[t, 0] = al[0, slot_off[s]: slot_off[s] + n]
                out[t, 1:4] = co[:, slot_off[s]: slot_off[s] + n].T
    return out
